# revision 63
# baseline (speedup 1.0000x reference)
"""Trainium2 Bass kernel for nn_AttentionHead (B=4, S=2048, H=D=1024, 8 cores).

Reference semantics (fp32):
    q = x @ Wq.T; k = x @ Wk.T; v = x @ Wv.T          (per batch b)
    kT = k.reshape(b, d, s)                            (raw reshape, NOT transpose)
    scores = q @ kT / sqrt(d)
    attn = softmax(scores, axis=0)                     (softmax over BATCH)
    attn_masked = where(tril(s, s), attn, 1e-9)
    out = attn_masked @ v

Sharding: every core computes k/v for a contiguous 256-row sequence shard and
the shards are exchanged with per-batch AllGathers (k first — scores only need
k; the v gathers overlap the scores phase).  The batch-softmax couples batches
at identical (i, j), so all 4 batches of a given attention-map tile live on
one core.  Scores are built transposed ([j, i]) so attn @ v needs no on-chip
transpose; kT = reshape(k) row tiles are plain strided DMA reads of the
gathered k.  The causal mask comes from a host-precomputed per-core mask
tensor, keeping the SPMD program identical on every core.

Precision: all matmuls run single-pass fp16 with fp32 PSUM accumulation
(5.8e-4 relative error on hardware, well inside the 2e-2 gate).  x and the
weights are rounded to fp16 on the host; k/v are gathered as fp16; the output
is returned as fp16 and upcast on the host.  The post-mask 1e-9 fill
contributes ~1e-9 relative to the output scale and is dropped entirely.

Causal staircase (16-row granularity): each core holds sixteen 16-row
sub-blocks {16k + c, 16k + 15 - c} (ascending), so for every core the first
io(jt) = 16*jt i-rows of j-tile jt are fully masked (io = 0 for jt <= 1).
Scores compute only the live suffix, the attn tiles store only the live
suffix, and the transposed attn@v accumulates [d-chunk, i-suffix] per j tile
— the matmul cost-model streaming dim is the suffix width, so masked work is
skipped in both phases.  attn@v runs jt0 first / jt1 last so the full psum
width is opened and closed by full-cover matmuls; the host transposes the
[d, i] output back.

Engine/DMA layout (cost-model driven):
  - matmul cost = out-free-size x 0.4167ns (fp16); DMA = 360 GB/s aggregate
    with >=512B contiguous runs; HWDGE descriptor generation is a serial
    ~625ns/DMA resource -> few large strided DMAs per stream;
  - the SP queue carries only WAR-free loads in consumption order (x, w
    chunks, k/v tile stream); WAR-gated loads (xq reusing x slots, wq) and
    dependency-gated stores issue on the Activation queue; k/v tiles stream
    as 4KB/8KB half-tiles so a small urgent store never queues long behind a
    bulk transfer, with pool-recycling WAR deps pacing the just-in-time tail;
  - persistent pools (q/k/v tiles) open before the projection pools so
    prefetch DMAs carry no WAR dep on recycled projection SBUF; attn tiles
    and the mask live in pools opened after the projection pools close
    (their writes start post-q-projection anyway), freeing peak SBUF for
    deeper k/v prefetch (11 + 9 half-tile buffers);
  - softmax work is spread: exp on Activation, den-sum on GpSimd, recip and
    the fused (e * mask/den -> fp16 attn tile) multiply on DVE; psum->SBUF
    copies on DVE; kv/q/out stores merged into wide panels.
"""
import numpy as np

B, S, H, D = 4, 2048, 1024, 1024
R = 8                  # cores
SL = S // R            # kv shard rows per core (contiguous)
IB = 128               # i block height
NJT = S // IB          # 16 j tiles of 128
ILOC = 2 * IB          # local q rows per core

_CACHE = {}


def _subrows(c):
    subs = []
    for k in range(8):
        subs += [16 * k + c, 16 * k + 15 - c]
    return np.concatenate([np.arange(16 * s, 16 * s + 16) for s in subs])


def _io(jt):
    # i-prefix of tile jt that is fully masked on every core (16-row slots)
    return 16 * jt if jt >= 2 else 0


def _build_program(sim=False):
    from contextlib import ExitStack

    import concourse.bacc as bacc
    import concourse.mybir as mybir
    from concourse import tile

    f32 = mybir.dt.float32
    f16 = mybir.dt.float16
    nc = bacc.Bacc("TRN2", target_bir_lowering=False, debug=False,
                   num_devices=(1 if sim else R))

    xt_q = nc.dram_tensor("xt_q", [B, H, ILOC], f16, kind="ExternalInput").ap()
    xt_kv = nc.dram_tensor("xt_kv", [B, H, SL], f16, kind="ExternalInput").ap()
    wqt = nc.dram_tensor("wqt", [H, D], f16, kind="ExternalInput").ap()
    wkt = nc.dram_tensor("wkt", [H, D], f16, kind="ExternalInput").ap()
    wvt = nc.dram_tensor("wvt", [H, D], f16, kind="ExternalInput").ap()
    m1 = nc.dram_tensor("m1", [NJT, IB, ILOC], f16, kind="ExternalInput").ap()
    out_loc = nc.dram_tensor("out_loc", [B, D, ILOC], f16, kind="ExternalOutput").ap()

    with tile.TileContext(nc) as tc, ExitStack() as ctx:
        dram = ctx.enter_context(tc.tile_pool(name="dram", bufs=1, space="DRAM"))
        agi_k = dram.tile([B, SL, D], f16)
        agi_v = dram.tile([B, SL, D], f16)
        if sim:
            ag_k = [nc.dram_tensor(f"ag_k{b}", [R, SL, D], f16,
                                   kind="ExternalInput").ap() for b in range(B)]
            ag_v = [nc.dram_tensor(f"ag_v{b}", [R, SL, D], f16,
                                   kind="ExternalInput").ap() for b in range(B)]
        else:
            ag_k = [dram.tile([R, SL, D], f16, name=f"ag_k{b}")
                    for b in range(B)]
            ag_v = [dram.tile([R, SL, D], f16, name=f"ag_v{b}")
                    for b in range(B)]

        def all_gather(src_ap, dst_tile):
            nc.gpsimd.collective_compute(
                "AllGather", mybir.AluOpType.bypass,
                replica_groups=[list(range(R))],
                ins=[src_ap], outs=[dst_tile.opt() if not sim else dst_tile],
            )

        # --- persistent pools FIRST: their (prefetch) DMA writes must not
        # inherit WAR deps on recycled projection-pool SBUF space -----------
        qt_pool = ctx.enter_context(tc.tile_pool(name="qt", bufs=4))
        ktpool = ctx.enter_context(tc.tile_pool(name="ktpool", bufs=11))
        vpool = ctx.enter_context(tc.tile_pool(name="vpool", bufs=9))

        qt_h = []
        # wv and x[b3] outlive the projection pools: the v projection of
        # batch 3 is deferred into the scores phase (nothing there reads v),
        # converting PE-bound front time into back-half bubble fill
        wpb = ctx.enter_context(tc.tile_pool(name="wpb", bufs=1))

        # ================= KV + Q projections (weights freed after) =========
        # Queue discipline: the SP queue carries only WAR-free loads so the
        # k/v prefetch stream never stalls behind a semaphore wait; the
        # WAR-gated loads (wq reusing wk's slot, xq reusing x slots) issue on
        # the Pool / Activation queues where an alloc stall blocks nothing.
        with tc.tile_pool(name="wpool", bufs=1) as wpool, \
             tc.tile_pool(name="xkv", bufs=4) as xpool, \
             tc.tile_pool(name="kvsb", bufs=4) as kvpool, \
             tc.tile_pool(name="pskv", bufs=4, space="PSUM") as pskv:
            w_h = {
                "wa": wpool.tile([128, 8, D], f16, tag="wa", name="wa"),
                "wb": wpb.tile([128, 8, D], f16, tag="wb", name="wb"),
                "wc": wpool.tile([128, 8, D], f16, tag="wc", name="wc"),
            }

            def load_w(slot, w, queue):
                # fp16 weights straight from DRAM, split in 2-ht chunks so the
                # first matmul does not wait for the whole 2MB transfer
                wr = w.rearrange("(t p) d -> p t d", p=128)
                for cch in range(4):
                    queue.dma_start(
                        w_h[slot][:, 2 * cch:2 * cch + 2, :],
                        wr[:, 2 * cch:2 * cch + 2, :],
                    )

            xkv_h = []

            def load_xkv(b):
                xh = xpool.tile([128, 8, SL], f16, tag="xkvh", name="xkvh")
                xr = xt_kv[b].rearrange("(t p) s -> p t s", p=128)
                nc.sync.dma_start(xh[:, 0:4, :], xr[:, 0:4, :])
                nc.sync.dma_start(xh[:, 4:8, :], xr[:, 4:8, :])
                xkv_h.append(xh)

            xq_h = []

            def load_xq(b):
                # reuses an x slot; WAR-gated on the v-pass of batch b, so it
                # issues on the Activation queue right after b's kv store
                xh = xpool.tile([128, 8, ILOC], f16, tag="xkvh", name="xqh")
                nc.scalar.dma_start(
                    xh[:], xt_q[b].rearrange("(t p) s -> p t s", p=128)
                )
                xq_h.append(xh)

            load_xkv(0)
            load_w("wa", wkt, nc.sync)
            for b in range(1, B):
                load_xkv(b)

            # k pass first so every k AllGather is in flight before the
            # v pass; scores (jt-outer) need all four.
            def proj_pass(agi, widx, gather_dst, post_b=None, bs=None):
                for b in (range(B) if bs is None else bs):
                    xh = xkv_h[b]
                    wh = w_h[widx]
                    for st in range(SL // 128):
                        hl = kvpool.tile([128, 1024], f16, tag="kvhl",
                                         name="kvhl")
                        for dblk in range(D // 512):
                            ps = pskv.tile([128, 512], f32, tag="pskv",
                                           name="pskv")
                            for ht in range(8):
                                nc.tensor.matmul(
                                    ps[:],
                                    xh[:, ht, st * 128:(st + 1) * 128],
                                    wh[:, ht, dblk * 512:(dblk + 1) * 512],
                                    start=(ht == 0),
                                    stop=(ht == 7),
                                )
                            nc.vector.tensor_copy(
                                hl[:, dblk * 512:(dblk + 1) * 512], ps[:]
                            )
                        nc.scalar.dma_start(
                            agi[b, st * 128:(st + 1) * 128, :], hl[:]
                        )
                    if post_b is not None:
                        post_b(b)
                    if not sim:
                        all_gather(agi[b], gather_dst[b])

            proj_pass(agi_k, "wa", ag_k)
            load_w("wb", wvt, nc.sync)
            load_w("wc", wqt, nc.sync)
            proj_pass(agi_v, "wb", ag_v, post_b=load_xq, bs=[0, 1, 2])
            load_xq(3)

            # ---- Q projection, stored transposed as fp16 -------------------
            with tc.tile_pool(name="psq", bufs=3, space="PSUM") as psq:
                for b in range(B):
                    xh = xq_h[b]
                    qh = qt_pool.tile([128, 8, ILOC], f16, tag="qth",
                                      name="qth")
                    qt_h.append(qh)
                    wh = w_h["wc"]
                    for mt in range(8):
                        ps = psq.tile([128, ILOC], f32, tag="psq", name="psq")
                        for ht in range(8):
                            nc.tensor.matmul(
                                ps[:],
                                wh[:, ht, mt * 128:(mt + 1) * 128],
                                xh[:, ht, :],
                                start=(ht == 0),
                                stop=(ht == 7),
                            )
                        nc.vector.tensor_copy(qh[:, mt, :], ps[:])

        # ---- k/v tile prefetch: all loads live on the SP queue, paced by
        # pool-recycling WAR deps; emission order interleaves the streams so
        # a stalled kt alloc never blocks the first v tiles -----------------
        kt_tiles = {}

        def load_kt(jtg, b):
            # two half-tiles (contraction panels 0-3 / 4-7): finer transfers
            # cap the latency a small urgent store can queue behind, and the
            # pool recycles mid-chain
            jh, chalf = jtg // 2, jtg % 2
            ksrc = ag_k[b].rearrange(
                "mt (p two) d -> two p mt d", two=2
            )[jh][:, :, chalf * 512:(chalf + 1) * 512]
            halves = []
            for h in range(2):
                kt = ktpool.tile([128, 4, 512], f16, tag="kt", name="kt")
                nc.sync.dma_start(kt[:], ksrc[:, 4 * h:4 * h + 4, :])
                halves.append(kt)
            kt_tiles[(jtg, b)] = halves

        vt_tiles = {}

        def load_vt(b, nblk):
            vsrc = ag_v[b].rearrange(
                "jh2 (jp p) d -> p jh2 jp d", jp=2
            )[:, :, :, nblk * 512:(nblk + 1) * 512]
            halves = []
            for h in range(2):
                vt = vpool.tile([128, 4, 2, 512], f16, tag="vt", name="vt")
                nc.sync.dma_start(vt[:, 0:2, :, :],
                                  vsrc[:, 4 * h:4 * h + 2, :, :])
                nc.sync.dma_start(vt[:, 2:4, :, :],
                                  vsrc[:, 4 * h + 2:4 * h + 4, :, :])
                halves.append(vt)
            vt_tiles[(b, nblk)] = halves

        for b in range(B):
            load_kt(0, b)
        load_kt(1, 0)
        load_vt(0, 0)
        load_vt(0, 1)
        load_vt(1, 0)
        load_vt(1, 1)
        for jtg in range(1, 4):
            for b in range(B):
                if (jtg, b) not in kt_tiles:
                    load_kt(jtg, b)
        for b in range(B):
            for nblk in range(2):
                if (b, nblk) not in vt_tiles:
                    load_vt(b, nblk)

        ahpool = ctx.enter_context(tc.tile_pool(name="ahpool", bufs=4))
        mpool = ctx.enter_context(tc.tile_pool(name="mpool", bufs=1))
        # attn tiles hold only the live causal i-suffix of each j tile: the
        # masked prefix is never read by the transposed attn@v matmuls
        ah_tiles = [[None] * NJT for _ in range(B)]
        for jt in range(NJT):
            w = ILOC - _io(jt)
            for bb in range(B):
                ah_tiles[bb][jt] = ahpool.tile([IB, w], f16, tag=f"ah{jt}",
                                               name=f"ah{jt}")
        m1_sb = mpool.tile([IB, NJT, ILOC], f16, tag="m1", name="m1")
        nc.scalar.dma_start(m1_sb[:], m1.rearrange("jt p i -> p jt i"))

        # ============== scores (transposed) + exp + batch softmax ===========
        # jt-outer: the batch-softmax of tile jt follows immediately, so the
        # rolling e-tile window stays small; attn tiles (fp16) persist.
        with tc.tile_pool(name="epool", bufs=16) as epool, \
             tc.tile_pool(name="smx", bufs=2) as smx, \
             tc.tile_pool(name="pss", bufs=4, space="PSUM") as pss:
            def v3_chain(st):
                # deferred v projection of batch 3 (one 128-row chain):
                # nothing in scores/attn@v reads v[b3] until its own attn@v
                # units, so these PE-bound chains fill vt-stream wait bubbles
                xh3 = x3_sb[0]
                wh3 = w_h["wb"]
                hl = hl3pool.tile([128, 1024], f16, tag="hl3", name="hl3")
                for dblk in range(D // 512):
                    ps = psv.tile([128, 512], f32, tag="ps3",
                                  name="ps3", bufs=2)
                    for ht in range(8):
                        nc.tensor.matmul(
                            ps[:],
                            xh3[:, ht, st * 128:(st + 1) * 128],
                            wh3[:, ht, dblk * 512:(dblk + 1) * 512],
                            start=(ht == 0),
                            stop=(ht == 7),
                        )
                    nc.vector.tensor_copy(
                        hl[:, dblk * 512:(dblk + 1) * 512], ps[:]
                    )
                nc.scalar.dma_start(
                    agi_v[3, st * 128:(st + 1) * 128, :], hl[:]
                )
                if st == SL // 128 - 1 and not sim:
                    all_gather(agi_v[3], ag_v[3])

            for jtg in range(4):              # groups of 4 j-tiles
                e_grp = {}
                for b in range(B):
                    ktA, ktB = kt_tiles.pop((jtg, b))
                    qh = qt_h[b]
                    for q in range(4):
                        jt = jtg * 4 + q
                        io = _io(jt)
                        w = ILOC - io
                        ps = pss.tile([128, w], f32, tag="pss", name="pss",
                                       bufs=3)
                        for mt in range(8):
                            kth = ktA if mt < 4 else ktB
                            nc.tensor.matmul(
                                ps[:],
                                kth[:, mt % 4, q * 128:(q + 1) * 128],
                                qh[:, mt, io:io + w],
                                start=(mt == 0),
                                stop=(mt == 7),
                            )
                        e = epool.tile([IB, ILOC], f32, tag="e", name="e")
                        nc.scalar.activation(
                            e[:, io:io + w], ps[:],
                            mybir.ActivationFunctionType.Exp,
                            scale=float(1.0 / np.sqrt(D)),
                        )
                        e_grp[(b, jt)] = e
                        if b < B - 1:
                            continue
                        # ---- softmax over batch + mask + fp16 --------------
                        den = smx.tile([IB, w], f32, tag="den", name="den")
                        nc.gpsimd.tensor_add(
                            den[:], e_grp[(0, jt)][:, io:io + w],
                            e_grp[(1, jt)][:, io:io + w]
                        )
                        nc.gpsimd.tensor_add(
                            den[:], den[:], e_grp[(2, jt)][:, io:io + w]
                        )
                        nc.gpsimd.tensor_add(
                            den[:], den[:], e_grp[(3, jt)][:, io:io + w]
                        )
                        rm = smx.tile([IB, w], f32, tag="rm", name="rm")
                        nc.vector.reciprocal(rm[:], den[:])
                        nc.vector.tensor_mul(rm[:], rm[:],
                                             m1_sb[:, jt, io:io + w])
                        for bb in range(B):
                            ah = ah_tiles[bb][jt]
                            nc.vector.tensor_mul(
                                ah[:], e_grp[(bb, jt)][:, io:io + w], rm[:]
                            )

            # ===================== attn @ v ===================================
            # Transposed output: psum [d-chunk 128, i] accumulated over j
            # tiles, each matmul covering only the live causal i-suffix
            # (rows = suffix width, the cost-model streaming dim).  jt=0 runs
            # first (start=True, full width) and jt=1 last (stop=True, full
            # width) so every psum column is opened/closed by a full-cover
            # matmul.  The host transposes [d, i] back to [i, d].
            with tc.tile_pool(name="opool", bufs=3) as opool, \
                 tc.tile_pool(name="hl3p", bufs=2) as hl3pool, \
                 tc.tile_pool(name="x3p", bufs=1) as x3pool, \
                 tc.tile_pool(name="psv", bufs=4, space="PSUM") as psv:
                x3_sb = [x3pool.tile([128, 8, SL], f16, tag="x3", name="x3")]
                nc.scalar.dma_start(
                    x3_sb[0][:],
                    xt_kv[3].rearrange("(t p) s -> p t s", p=128),
                )
                jt_order = [0] + list(range(2, NJT)) + [1]
                unit = 0
                for b in range(B):
                    for nblk in range(D // 512):
                        if unit in (1, 2):
                            v3_chain(unit - 1)
                        unit += 1
                        # the final unit's stores issue on the idle SP queue
                        # (all its loads are done) instead of queueing on Act
                        oq = nc.sync if unit >= 7 else nc.scalar
                        vtA, vtB = vt_tiles.pop((b, nblk))
                        for dg in range(2):       # 2 d-chunks of 128 per nblk
                            osb = opool.tile([128, 2, ILOC], f16, tag="osb",
                                             name="osb")
                            for dc in range(2):
                                ps = psv.tile([128, ILOC], f32, bufs=3, tag="pv",
                                              name="pv")
                                dlo = (2 * dg + dc) * 128
                                for idx, jt in enumerate(jt_order):
                                    io = _io(jt)
                                    w = ILOC - io
                                    vth = vtA if jt < 8 else vtB
                                    vh = vth[:, (jt % 8) // 2, jt % 2,
                                             dlo:dlo + 128]
                                    ah = ah_tiles[b][jt]
                                    nc.tensor.matmul(
                                        ps[:, io:io + w], vh, ah[:],
                                        start=(idx == 0),
                                        stop=(idx == NJT - 1),
                                    )
                                nc.vector.tensor_copy(osb[:, dc, :], ps[:])
                            oq.dma_start(
                                out_loc[b].rearrange(
                                    "(g t p) i -> g p t i", g=4, t=2
                                )[2 * nblk + dg],
                                osb[:],
                            )

    nc.compile()
    return nc


def _host_inputs(x, Wq, Wk, Wv):
    x = np.asarray(x, dtype=np.float32)
    x16 = x.astype(np.float16)
    wqt = np.ascontiguousarray(np.asarray(Wq, dtype=np.float32).T
                               .astype(np.float16))
    wkt = np.ascontiguousarray(np.asarray(Wk, dtype=np.float32).T
                               .astype(np.float16))
    wvt = np.ascontiguousarray(np.asarray(Wv, dtype=np.float32).T
                               .astype(np.float16))

    in_maps = []
    for c in range(R):
        rows = _subrows(c)
        xt_q = np.ascontiguousarray(x16[:, rows, :].transpose(0, 2, 1))
        xt_kv = np.ascontiguousarray(
            x16[:, c * SL:(c + 1) * SL, :].transpose(0, 2, 1)
        )
        gi = rows[None, None, :]                       # global i (1,1,ILOC)
        jj = (np.arange(NJT)[:, None, None] * IB
              + np.arange(IB)[None, :, None])          # global j (NJT,IB,1)
        m1 = (jj <= gi).astype(np.float16)
        in_maps.append({
            "xt_q": xt_q, "xt_kv": xt_kv,
            "wqt": wqt, "wkt": wkt, "wvt": wvt,
            "m1": np.ascontiguousarray(m1),
        })
    return in_maps


def kernel(x, Wq, Wk, Wv):
    from concourse.bass_utils import run_bass_kernel_spmd

    if "nc" not in _CACHE:
        _CACHE["nc"] = _build_program()
    nc = _CACHE["nc"]

    in_maps = _host_inputs(x, Wq, Wk, Wv)
    res = None
    for attempt in range(3):
        try:
            res = run_bass_kernel_spmd(nc, in_maps, list(range(R)))
            break
        except Exception:
            # transient NRT_EXEC_UNIT_UNRECOVERABLE wedges recover on retry
            if attempt == 2:
                raise
            import time
            time.sleep(15)

    out = np.empty((B, S, D), dtype=np.float32)
    for c in range(R):
        out[:, _subrows(c), :] = (res.results[c]["out_loc"]
                                  .astype(np.float32).transpose(0, 2, 1))
    return out


if __name__ == "__main__":
    rng = np.random.default_rng(0)
    x = rng.standard_normal((B, S, H), dtype=np.float32)
    Wq = rng.standard_normal((D, H), dtype=np.float32) / np.sqrt(H)
    Wk = rng.standard_normal((D, H), dtype=np.float32) / np.sqrt(H)
    Wv = rng.standard_normal((D, H), dtype=np.float32) / np.sqrt(H)
    o = kernel(x, Wq, Wk, Wv)
    print("kernel output", o.shape, o.dtype, float(np.abs(o).max()))
